# revision 3
# baseline (speedup 1.0000x reference)
"""RWKV TimeMix kernel for Trainium2, 8 NeuronCores.

Sharding (per spec hint): data-parallel over B (4 batches) x tensor-parallel
over the channel dim (2 halves of C=1024). Core i handles batch i//2 and
output-channel half i%2 for the full T=8192 sequence, so the WKV cumsum
(channelwise independent) never crosses cores. Each core returns a partial
y (its channel half's contribution through Wo); the host gather sums the
two partials per batch.

Per-core device pipeline (all layouts channel-on-partitions after an
on-chip PE transpose):
  LayerNorm (affine folded into weights on host) -> xx
  xx^T via PE transpose -> k,v,r projections (PSUM accum over 8 c-chunks)
  kvw = k*v*exp(td*t) (decay weights host-precomputed, DMA'd)
  hist = exclusive cumsum_t(kvw) via DVE tensor_tensor_scan (chained chunks)
  wkv = hist + k*tf ; og = sigmoid(r)*wkv
  y_part = og @ WoT (og^T blocks stationary -> output lands t-major, DMA out)
"""

import os
from contextlib import ExitStack

import numpy as np

import concourse.bass as bass
import concourse.mybir as mybir
import concourse.tile as tile
from concourse.masks import make_identity

F32 = mybir.dt.float32
BF16 = mybir.dt.bfloat16
AF = mybir.ActivationFunctionType
OP = mybir.AluOpType

B, T, C = 4, 8192, 1024
DL = C // 2
TC = 512
EPS = 1e-5

MM_DT = F32  # matmul operand dtype

# ---------------------------------------------------------------------------
# Workaround for the walrus build here, which accepts only ONE semaphore wait
# per instruction: split multi-waits onto same-engine NoOp carriers, and move
# the end-of-kernel Drain's waits onto single-wait SP nops.
_split_counter = [0]


def _split_waits_in_list(nc, insts):
    out = []
    for inst in insts:
        si = inst.sync_info
        if si is not None and len(si.on_wait) > 1:
            waits = list(si.on_wait)
            si.on_wait = waits[:1]
            for w in waits[1:]:
                _split_counter[0] += 1
                nop = mybir.InstNoOp(name=f"wsplit_{_split_counter[0]}")
                nop.engine = inst.engine
                nop.sync_info = mybir.SyncInfo(on_wait=[w], on_update=[])
                nop.debug = inst.debug
                nc.register_instruction(nop)
                out.append(nop)
        out.append(inst)
    return out


_orig_lower = tile.TileContext._lower_ordered_insts


def _patched_lower_ordered_insts(self, ordered):
    for bb_name in list(ordered.keys()):
        ordered[bb_name] = _split_waits_in_list(self.nc, ordered[bb_name])
    return _orig_lower(self, ordered)


def _patched_drain_and_barrier(self, tick_clock, wait_clock):
    from concourse.vector_clock import ScopedClock

    drain_inst = self.nc.sync.drain()
    wait_clock.add_sem_waits(
        drain_inst.ins, ScopedClock({None: tick_clock.global_clock})
    )
    waits = list(drain_inst.ins.sync_info.on_wait)
    if len(waits) > 1:
        drain_inst.ins.sync_info.on_wait = []
        for w in waits:
            nop_ins = self.nc.sync.nop(nofuse=True, hint="drain_wait_split")
            nop_ins.ins.sync_info = mybir.SyncInfo(on_wait=[w], on_update=[])

    self.nc.all_engine_barrier()
    assert self.sems is not None
    popped = self.nc._tile_sem_poison_stack.pop()
    assert popped is self._sem_poison
    self.nc.clear_and_free_semaphores(list(self.sems.allocated().values()))
    self.nc.all_engine_barrier()


tile.TileContext._lower_ordered_insts = _patched_lower_ordered_insts
tile.TileContext._drain_and_barrier = _patched_drain_and_barrier
# ---------------------------------------------------------------------------


def build_rwkv_nc(T, C, DL, TC, mm_dt=F32, num_devices=8, eps=1e-5):
    assert T % TC == 0 and TC % 128 == 0 and C % 128 == 0 and DL % 128 == 0
    CC = C // 128   # contraction chunks (full channel dim)
    DC = DL // 128  # this core's channel chunks
    NT = T // TC    # time chunks
    RT = TC // 128  # row tiles (128 timesteps) per chunk
    gs = min(C, 512)
    ng = C // gs
    assert C % gs == 0
    ych = min(C, 512)
    nyc = C // ych

    nc = bass.Bass(
        "TRN2", target_bir_lowering=False, debug=False, num_devices=num_devices
    )

    x_e = nc.declare_dram_parameter("x", [T, C], F32, isOutput=False)
    wmat_e = nc.declare_dram_parameter("wmat", [DL, T], F32, isOutput=False)
    wk_e = nc.declare_dram_parameter("wkT", [C, DL], mm_dt, isOutput=False)
    wv_e = nc.declare_dram_parameter("wvT", [C, DL], mm_dt, isOutput=False)
    wr_e = nc.declare_dram_parameter("wrT", [C, DL], mm_dt, isOutput=False)
    wo_e = nc.declare_dram_parameter("woT", [DL, C], mm_dt, isOutput=False)
    tf_e = nc.declare_dram_parameter("tf", [DL], F32, isOutput=False)
    y_e = nc.declare_dram_parameter("y", [T, C], F32, isOutput=True)

    with tile.TileContext(nc) as tc, ExitStack() as ctx:
        singles = ctx.enter_context(tc.tile_pool(name="singles", bufs=1))
        xin = ctx.enter_context(tc.tile_pool(name="xin", bufs=3))
        xnorm = ctx.enter_context(tc.tile_pool(name="xnorm", bufs=2))
        stats = ctx.enter_context(tc.tile_pool(name="stats", bufs=4))
        xt = ctx.enter_context(tc.tile_pool(name="xt", bufs=2))
        wdec = ctx.enter_context(tc.tile_pool(name="wdec", bufs=DC + 2))
        act1 = ctx.enter_context(tc.tile_pool(name="act1", bufs=3))
        kvp = ctx.enter_context(tc.tile_pool(name="kvp", bufs=1))
        hist = ctx.enter_context(tc.tile_pool(name="hist", bufs=2))
        mid = ctx.enter_context(tc.tile_pool(name="mid", bufs=3))
        ogp = ctx.enter_context(tc.tile_pool(name="ogp", bufs=2))
        yout = ctx.enter_context(tc.tile_pool(name="yout", bufs=2))
        # PSUM: 8 banks. ps_a holds transposes (stage A) + y pieces (stage C)
        # on a shared tag; k and v/r get their own pools.
        psum_a = ctx.enter_context(tc.tile_pool(name="psum_a", bufs=3, space="PSUM"))
        psum_k = ctx.enter_context(tc.tile_pool(name="psum_k", bufs=2, space="PSUM"))
        psum_vr = ctx.enter_context(tc.tile_pool(name="psum_vr", bufs=3, space="PSUM"))

        ident = singles.tile([128, 128], F32)
        make_identity(nc, ident[:])
        eps_t = singles.tile([128, 1], F32)
        nc.vector.memset(eps_t[:], float(eps))

        wk_s = singles.tile([128, CC, DL], mm_dt)
        nc.sync.dma_start(wk_s[:], wk_e.rearrange("(cc p) d -> p cc d", p=128))
        wv_s = singles.tile([128, CC, DL], mm_dt)
        nc.sync.dma_start(wv_s[:], wv_e.rearrange("(cc p) d -> p cc d", p=128))
        wr_s = singles.tile([128, CC, DL], mm_dt)
        nc.sync.dma_start(wr_s[:], wr_e.rearrange("(cc p) d -> p cc d", p=128))
        wo_s = singles.tile([128, DC, C], mm_dt)
        nc.sync.dma_start(wo_s[:], wo_e.rearrange("(dc p) d -> p dc d", p=128))
        tf_s = singles.tile([128, DC], F32)
        nc.sync.dma_start(tf_s[:], tf_e.rearrange("(dc p) -> p dc", p=128))

        kvw_bufs = [
            kvp.tile([128, TC + 1], F32, tag=f"kvw{dd}", name=f"kvw{dd}")
            for dd in range(DC)
        ]
        for dd in range(DC):
            nc.vector.memset(kvw_bufs[dd][:, 0:1], 0.0)

        prev_hist = [None] * DC

        for ch in range(NT):
            # ---- Stage A: layernorm + transpose -> xxT (c on partitions)
            xxT = xt.tile([128, CC, TC], mm_dt, tag="xxT")
            for r in range(RT):
                row0 = ch * TC + r * 128
                x_t = xin.tile([128, C], F32, tag="x")
                nc.sync.dma_start(x_t[:], x_e[row0 : row0 + 128, :])
                st = stats.tile([128, ng, 6], F32, tag="bnst")
                for gi in range(ng):
                    nc.vector.bn_stats(st[:, gi, :], x_t[:, gi * gs : (gi + 1) * gs])
                mv = stats.tile([128, 2], F32, tag="mv")
                nc.vector.bn_aggr(mv[:], st[:])
                rstd = stats.tile([128, 1], F32, tag="rstd")
                nc.scalar.activation(
                    rstd[:], mv[:, 1:2], AF.Sqrt, bias=eps_t[:, 0:1], scale=1.0
                )
                nc.vector.reciprocal(rstd[:], rstd[:])
                nb = stats.tile([128, 1], F32, tag="nb")
                nc.vector.tensor_scalar(
                    out=nb[:],
                    in0=mv[:, 0:1],
                    scalar1=rstd[:, 0:1],
                    scalar2=-1.0,
                    op0=OP.mult,
                    op1=OP.mult,
                )
                xxn = xnorm.tile([128, C], F32, tag="xxn")
                nc.scalar.activation(
                    xxn[:], x_t[:], AF.Identity, bias=nb[:, 0:1], scale=rstd[:, 0:1]
                )
                for cc in range(CC):
                    trp = psum_a.tile([128, 128], F32, tag="ps_a")
                    nc.tensor.transpose(
                        trp[:], xxn[:, cc * 128 : (cc + 1) * 128], ident[:]
                    )
                    nc.scalar.copy(xxT[:, cc, r * 128 : (r + 1) * 128], trp[:])

            # ---- Stage B: projections + wkv recurrence per channel chunk
            og_tiles = []
            for dd in range(DC):
                w_t = wdec.tile([128, TC], F32, tag="wdec")
                nc.sync.dma_start(
                    w_t[:], wmat_e[dd * 128 : (dd + 1) * 128, ch * TC : (ch + 1) * TC]
                )
                k_ps = psum_k.tile([128, TC], F32, tag="kps")
                v_ps = psum_vr.tile([128, TC], F32, tag="vrps")
                r_ps = psum_vr.tile([128, TC], F32, tag="vrps")
                d_sl = slice(dd * 128, (dd + 1) * 128)
                for cc in range(CC):
                    first, last = cc == 0, cc == CC - 1
                    nc.tensor.matmul(
                        k_ps[:], lhsT=wk_s[:, cc, d_sl], rhs=xxT[:, cc, :],
                        start=first, stop=last,
                    )
                    nc.tensor.matmul(
                        v_ps[:], lhsT=wv_s[:, cc, d_sl], rhs=xxT[:, cc, :],
                        start=first, stop=last,
                    )
                    nc.tensor.matmul(
                        r_ps[:], lhsT=wr_s[:, cc, d_sl], rhs=xxT[:, cc, :],
                        start=first, stop=last,
                    )
                g_t = act1.tile([128, TC], F32, tag="g")
                nc.scalar.activation(g_t[:], r_ps[:], AF.Sigmoid)
                v_t = act1.tile([128, TC], F32, tag="v")
                nc.scalar.copy(v_t[:], v_ps[:])
                kv_t = mid.tile([128, TC], F32, tag="kv")
                nc.vector.tensor_mul(kv_t[:], k_ps[:], v_t[:])
                kvb = kvw_bufs[dd]
                if ch > 0:
                    nc.gpsimd.tensor_copy(out=kvb[:, 0:1], in_=kvb[:, TC : TC + 1])
                nc.vector.tensor_mul(kvb[:, 1 : TC + 1], kv_t[:], w_t[:])
                h_t = hist.tile([128, TC], F32, tag=f"hist{dd}")
                init = 0.0 if ch == 0 else prev_hist[dd][:, TC - 1 : TC]
                nc.vector.tensor_tensor_scan(
                    out=h_t[:],
                    data0=kvb[:, 0:TC],
                    data1=kvb[:, 0:TC],
                    initial=init,
                    op0=OP.add,
                    op1=OP.bypass,
                )
                prev_hist[dd] = h_t
                wkv_t = mid.tile([128, TC], F32, tag="wkv")
                nc.vector.scalar_tensor_tensor(
                    out=wkv_t[:],
                    in0=k_ps[:],
                    scalar=tf_s[:, dd : dd + 1],
                    in1=h_t[:],
                    op0=OP.mult,
                    op1=OP.add,
                )
                og_t = ogp.tile([128, TC], mm_dt, tag=f"og{dd}")
                nc.vector.tensor_mul(og_t[:], wkv_t[:], g_t[:])
                og_tiles.append(og_t)

            # ---- Stage C: y_partial = og @ WoT, output in natural layout
            for tb in range(RT):
                y_sb = yout.tile([128, C], F32, tag="y")
                t_sl = slice(tb * 128, (tb + 1) * 128)
                for h in range(nyc):
                    y_ps = psum_a.tile([128, ych], F32, tag="ps_a")
                    c_sl = slice(h * ych, (h + 1) * ych)
                    for dd in range(DC):
                        nc.tensor.matmul(
                            y_ps[:],
                            lhsT=og_tiles[dd][:, t_sl],
                            rhs=wo_s[:, dd, c_sl],
                            start=(dd == 0),
                            stop=(dd == DC - 1),
                        )
                    nc.scalar.copy(y_sb[:, c_sl], y_ps[:])
                row0 = ch * TC + tb * 128
                nc.sync.dma_start(y_e[row0 : row0 + 128, :], y_sb[:])

    return nc


_np_mm = {F32: np.float32, BF16: np.float32}  # host arrays stay f32; see below


def _mm_np(a, mm_dt):
    if mm_dt == BF16:
        import ml_dtypes

        return np.asarray(a, dtype=ml_dtypes.bfloat16)
    return np.asarray(a, dtype=np.float32)


_cached = {}


def _get_nc(mm_dt):
    key = (T, C, DL, TC, mm_dt)
    if key not in _cached:
        _cached[key] = build_rwkv_nc(T, C, DL, TC, mm_dt=mm_dt, num_devices=8)
    return _cached[key]


def kernel(x, time_decay, time_first, Wk, Wv, Wr, Wo, ln_w, ln_b):
    from concourse.bass_utils import run_bass_kernel_spmd

    x = np.asarray(x, dtype=np.float32)
    time_decay = np.asarray(time_decay, dtype=np.float32)
    time_first = np.asarray(time_first, dtype=np.float32)
    Wk = np.asarray(Wk, dtype=np.float32)
    Wv = np.asarray(Wv, dtype=np.float32)
    Wr = np.asarray(Wr, dtype=np.float32)
    Wo = np.asarray(Wo, dtype=np.float32)
    ln_w = np.asarray(ln_w, dtype=np.float32)
    ln_b = np.asarray(ln_b, dtype=np.float32)
    assert x.shape == (B, T, C)
    if np.any(ln_b != 0.0):
        raise NotImplementedError("nonzero LayerNorm bias not supported")

    mm_dt = MM_DT
    nc = _get_nc(mm_dt)

    td = -np.exp(time_decay)                  # (C,)
    tf = np.exp(time_first)                   # (C,)
    tpos = np.arange(T, dtype=np.float32)
    WkT = np.ascontiguousarray((Wk * ln_w[None, :]).T)  # (C, C) = (c_in, d)
    WvT = np.ascontiguousarray((Wv * ln_w[None, :]).T)
    WrT = np.ascontiguousarray((Wr * ln_w[None, :]).T)
    WoT = np.ascontiguousarray(Wo.T)                    # (c, d)

    half_maps = []
    with np.errstate(under="ignore"):
        for dh in range(2):
            d0, d1 = dh * DL, (dh + 1) * DL
            half_maps.append(
                {
                    "wmat": np.exp(td[d0:d1, None] * tpos[None, :]).astype(
                        np.float32
                    ),
                    "wkT": _mm_np(WkT[:, d0:d1], mm_dt),
                    "wvT": _mm_np(WvT[:, d0:d1], mm_dt),
                    "wrT": _mm_np(WrT[:, d0:d1], mm_dt),
                    "woT": _mm_np(WoT[d0:d1, :], mm_dt),
                    "tf": np.ascontiguousarray(tf[d0:d1]),
                }
            )
    in_maps = []
    for core in range(8):
        b, dh = core // 2, core % 2
        in_maps.append({"x": np.ascontiguousarray(x[b]), **half_maps[dh]})

    global _last_in_maps
    _last_in_maps = in_maps
    res = run_bass_kernel_spmd(nc, in_maps, list(range(8)))
    y = np.empty((B, T, C), dtype=np.float32)
    for b in range(B):
        y[b] = res.results[2 * b]["y"] + res.results[2 * b + 1]["y"]
    return y


# revision 4
# speedup vs baseline: 1.4275x; 1.4275x over previous
"""RWKV TimeMix kernel for Trainium2, 8 NeuronCores.

Sharding (per spec hint): data-parallel over B (4 batches) x tensor-parallel
over the channel dim (2 halves of C=1024). Core i handles batch i//2 and
output-channel half i%2 for the full T=8192 sequence, so the WKV cumsum
(channelwise independent) never crosses cores. Each core returns a partial
y (its channel half's contribution through Wo); the host gather sums the
two partials per batch.

Per-core device pipeline (all layouts channel-on-partitions after an
on-chip PE transpose):
  LayerNorm (affine folded into weights on host) -> xx
  xx^T via PE transpose -> k,v,r projections (PSUM accum over 8 c-chunks)
  kvw = k*v*exp(td*t) (decay weights host-precomputed, DMA'd)
  hist = exclusive cumsum_t(kvw) via DVE tensor_tensor_scan (chained chunks)
  wkv = hist + k*tf ; og = sigmoid(r)*wkv
  y_part = og @ WoT (og^T blocks stationary -> output lands t-major, DMA out)
"""

import os
from contextlib import ExitStack

import numpy as np

import concourse.bass as bass
import concourse.mybir as mybir
import concourse.tile as tile
from concourse.masks import make_identity

F32 = mybir.dt.float32
BF16 = mybir.dt.bfloat16
AF = mybir.ActivationFunctionType
OP = mybir.AluOpType

B, T, C = 4, 8192, 1024
DL = C // 2
TC = 512
EPS = 1e-5

MM_DT = BF16  # matmul operand dtype

# ---------------------------------------------------------------------------
# Workaround for the walrus build here, which accepts only ONE semaphore wait
# per instruction: split multi-waits onto same-engine NoOp carriers, and move
# the end-of-kernel Drain's waits onto single-wait SP nops.
_split_counter = [0]


def _split_waits_in_list(nc, insts):
    out = []
    for inst in insts:
        si = inst.sync_info
        if si is not None and len(si.on_wait) > 1:
            waits = list(si.on_wait)
            si.on_wait = waits[:1]
            for w in waits[1:]:
                _split_counter[0] += 1
                nop = mybir.InstNoOp(name=f"wsplit_{_split_counter[0]}")
                nop.engine = inst.engine
                nop.sync_info = mybir.SyncInfo(on_wait=[w], on_update=[])
                nop.debug = inst.debug
                nc.register_instruction(nop)
                out.append(nop)
        out.append(inst)
    return out


_orig_lower = tile.TileContext._lower_ordered_insts


def _patched_lower_ordered_insts(self, ordered):
    for bb_name in list(ordered.keys()):
        ordered[bb_name] = _split_waits_in_list(self.nc, ordered[bb_name])
    return _orig_lower(self, ordered)


def _patched_drain_and_barrier(self, tick_clock, wait_clock):
    from concourse.vector_clock import ScopedClock

    drain_inst = self.nc.sync.drain()
    wait_clock.add_sem_waits(
        drain_inst.ins, ScopedClock({None: tick_clock.global_clock})
    )
    waits = list(drain_inst.ins.sync_info.on_wait)
    if len(waits) > 1:
        drain_inst.ins.sync_info.on_wait = []
        for w in waits:
            nop_ins = self.nc.sync.nop(nofuse=True, hint="drain_wait_split")
            nop_ins.ins.sync_info = mybir.SyncInfo(on_wait=[w], on_update=[])

    self.nc.all_engine_barrier()
    assert self.sems is not None
    popped = self.nc._tile_sem_poison_stack.pop()
    assert popped is self._sem_poison
    self.nc.clear_and_free_semaphores(list(self.sems.allocated().values()))
    self.nc.all_engine_barrier()


tile.TileContext._lower_ordered_insts = _patched_lower_ordered_insts
tile.TileContext._drain_and_barrier = _patched_drain_and_barrier
# ---------------------------------------------------------------------------


def build_rwkv_nc(T, C, DL, TC, mm_dt=F32, num_devices=8, eps=1e-5):
    assert T % TC == 0 and TC % 128 == 0 and C % 128 == 0 and DL % 128 == 0
    CC = C // 128   # contraction chunks (full channel dim)
    DC = DL // 128  # this core's channel chunks
    NT = T // TC    # time chunks
    RT = TC // 128  # row tiles (128 timesteps) per chunk
    gs = min(C, 512)
    ng = C // gs
    assert C % gs == 0
    ych = min(C, 512)
    nyc = C // ych

    nc = bass.Bass(
        "TRN2", target_bir_lowering=False, debug=False, num_devices=num_devices
    )

    x_e = nc.declare_dram_parameter("x", [T, C], F32, isOutput=False)
    wmat_e = nc.declare_dram_parameter("wmat", [DL, T], F32, isOutput=False)
    wk_e = nc.declare_dram_parameter("wkT", [C, DL], mm_dt, isOutput=False)
    wv_e = nc.declare_dram_parameter("wvT", [C, DL], mm_dt, isOutput=False)
    wr_e = nc.declare_dram_parameter("wrT", [C, DL], mm_dt, isOutput=False)
    wo_e = nc.declare_dram_parameter("woT", [DL, C], mm_dt, isOutput=False)
    tf_e = nc.declare_dram_parameter("tf", [DL], F32, isOutput=False)
    y_e = nc.declare_dram_parameter("y", [T, C], F32, isOutput=True)

    with tile.TileContext(nc) as tc, ExitStack() as ctx:
        singles = ctx.enter_context(tc.tile_pool(name="singles", bufs=1))
        xin = ctx.enter_context(tc.tile_pool(name="xin", bufs=3))
        xnorm = ctx.enter_context(tc.tile_pool(name="xnorm", bufs=2))
        stats = ctx.enter_context(tc.tile_pool(name="stats", bufs=4))
        xt = ctx.enter_context(tc.tile_pool(name="xt", bufs=2))
        wdec = ctx.enter_context(tc.tile_pool(name="wdec", bufs=DC + 2))
        act1 = ctx.enter_context(tc.tile_pool(name="act1", bufs=3))
        kvp = ctx.enter_context(tc.tile_pool(name="kvp", bufs=1))
        hist = ctx.enter_context(tc.tile_pool(name="hist", bufs=2))
        mid = ctx.enter_context(tc.tile_pool(name="mid", bufs=3))
        ogp = ctx.enter_context(tc.tile_pool(name="ogp", bufs=2))
        yout = ctx.enter_context(tc.tile_pool(name="yout", bufs=2))
        # PSUM: 8 banks. ps_a holds transposes (stage A) + y pieces (stage C)
        # on a shared tag; k and v/r get their own pools.
        psum_a = ctx.enter_context(tc.tile_pool(name="psum_a", bufs=3, space="PSUM"))
        psum_k = ctx.enter_context(tc.tile_pool(name="psum_k", bufs=2, space="PSUM"))
        psum_vr = ctx.enter_context(tc.tile_pool(name="psum_vr", bufs=3, space="PSUM"))

        ident = singles.tile([128, 128], F32)
        make_identity(nc, ident[:])
        eps_t = singles.tile([128, 1], F32)
        nc.vector.memset(eps_t[:], float(eps))

        wk_s = singles.tile([128, CC, DL], mm_dt)
        nc.sync.dma_start(wk_s[:], wk_e.rearrange("(cc p) d -> p cc d", p=128))
        wv_s = singles.tile([128, CC, DL], mm_dt)
        nc.sync.dma_start(wv_s[:], wv_e.rearrange("(cc p) d -> p cc d", p=128))
        wr_s = singles.tile([128, CC, DL], mm_dt)
        nc.sync.dma_start(wr_s[:], wr_e.rearrange("(cc p) d -> p cc d", p=128))
        wo_s = singles.tile([128, DC, C], mm_dt)
        nc.sync.dma_start(wo_s[:], wo_e.rearrange("(dc p) d -> p dc d", p=128))
        tf_s = singles.tile([128, DC], F32)
        nc.sync.dma_start(tf_s[:], tf_e.rearrange("(dc p) -> p dc", p=128))

        kvw_bufs = [
            kvp.tile([128, TC + 1], F32, tag=f"kvw{dd}", name=f"kvw{dd}")
            for dd in range(DC)
        ]
        for dd in range(DC):
            nc.vector.memset(kvw_bufs[dd][:, 0:1], 0.0)

        prev_hist = [None] * DC

        for ch in range(NT):
            # ---- Stage A: layernorm + transpose -> xxT (c on partitions)
            xxT = xt.tile([128, CC, TC], mm_dt, tag="xxT")
            for r in range(RT):
                row0 = ch * TC + r * 128
                x_t = xin.tile([128, C], F32, tag="x")
                nc.sync.dma_start(x_t[:], x_e[row0 : row0 + 128, :])
                st = stats.tile([128, ng, 6], F32, tag="bnst")
                for gi in range(ng):
                    nc.vector.bn_stats(st[:, gi, :], x_t[:, gi * gs : (gi + 1) * gs])
                mv = stats.tile([128, 2], F32, tag="mv")
                nc.vector.bn_aggr(mv[:], st[:])
                rstd = stats.tile([128, 1], F32, tag="rstd")
                nc.scalar.activation(
                    rstd[:], mv[:, 1:2], AF.Sqrt, bias=eps_t[:, 0:1], scale=1.0
                )
                nc.vector.reciprocal(rstd[:], rstd[:])
                nb = stats.tile([128, 1], F32, tag="nb")
                nc.vector.tensor_scalar(
                    out=nb[:],
                    in0=mv[:, 0:1],
                    scalar1=rstd[:, 0:1],
                    scalar2=-1.0,
                    op0=OP.mult,
                    op1=OP.mult,
                )
                xxn = xnorm.tile([128, C], F32, tag="xxn")
                nc.scalar.activation(
                    xxn[:], x_t[:], AF.Identity, bias=nb[:, 0:1], scale=rstd[:, 0:1]
                )
                for cc in range(CC):
                    trp = psum_a.tile([128, 128], F32, tag="ps_a")
                    nc.tensor.transpose(
                        trp[:], xxn[:, cc * 128 : (cc + 1) * 128], ident[:]
                    )
                    nc.scalar.copy(xxT[:, cc, r * 128 : (r + 1) * 128], trp[:])

            # ---- Stage B: projections + wkv recurrence per channel chunk
            og_tiles = []
            for dd in range(DC):
                w_t = wdec.tile([128, TC], F32, tag="wdec")
                nc.sync.dma_start(
                    w_t[:], wmat_e[dd * 128 : (dd + 1) * 128, ch * TC : (ch + 1) * TC]
                )
                k_ps = psum_k.tile([128, TC], F32, tag="kps")
                v_ps = psum_vr.tile([128, TC], F32, tag="vrps")
                r_ps = psum_vr.tile([128, TC], F32, tag="vrps")
                d_sl = slice(dd * 128, (dd + 1) * 128)
                for cc in range(CC):
                    first, last = cc == 0, cc == CC - 1
                    nc.tensor.matmul(
                        k_ps[:], lhsT=wk_s[:, cc, d_sl], rhs=xxT[:, cc, :],
                        start=first, stop=last,
                    )
                    nc.tensor.matmul(
                        v_ps[:], lhsT=wv_s[:, cc, d_sl], rhs=xxT[:, cc, :],
                        start=first, stop=last,
                    )
                    nc.tensor.matmul(
                        r_ps[:], lhsT=wr_s[:, cc, d_sl], rhs=xxT[:, cc, :],
                        start=first, stop=last,
                    )
                g_t = act1.tile([128, TC], F32, tag="g")
                nc.scalar.activation(g_t[:], r_ps[:], AF.Sigmoid)
                v_t = act1.tile([128, TC], F32, tag="v")
                nc.scalar.copy(v_t[:], v_ps[:])
                kv_t = mid.tile([128, TC], F32, tag="kv")
                nc.vector.tensor_mul(kv_t[:], k_ps[:], v_t[:])
                kvb = kvw_bufs[dd]
                if ch > 0:
                    nc.gpsimd.tensor_copy(out=kvb[:, 0:1], in_=kvb[:, TC : TC + 1])
                nc.vector.tensor_mul(kvb[:, 1 : TC + 1], kv_t[:], w_t[:])
                h_t = hist.tile([128, TC], F32, tag=f"hist{dd}")
                init = 0.0 if ch == 0 else prev_hist[dd][:, TC - 1 : TC]
                nc.vector.tensor_tensor_scan(
                    out=h_t[:],
                    data0=kvb[:, 0:TC],
                    data1=kvb[:, 0:TC],
                    initial=init,
                    op0=OP.add,
                    op1=OP.bypass,
                )
                prev_hist[dd] = h_t
                wkv_t = mid.tile([128, TC], F32, tag="wkv")
                nc.vector.scalar_tensor_tensor(
                    out=wkv_t[:],
                    in0=k_ps[:],
                    scalar=tf_s[:, dd : dd + 1],
                    in1=h_t[:],
                    op0=OP.mult,
                    op1=OP.add,
                )
                og_t = ogp.tile([128, TC], mm_dt, tag=f"og{dd}")
                nc.vector.tensor_mul(og_t[:], wkv_t[:], g_t[:])
                og_tiles.append(og_t)

            # ---- Stage C: y_partial = og @ WoT, output in natural layout
            for tb in range(RT):
                y_sb = yout.tile([128, C], F32, tag="y")
                t_sl = slice(tb * 128, (tb + 1) * 128)
                for h in range(nyc):
                    y_ps = psum_a.tile([128, ych], F32, tag="ps_a")
                    c_sl = slice(h * ych, (h + 1) * ych)
                    for dd in range(DC):
                        nc.tensor.matmul(
                            y_ps[:],
                            lhsT=og_tiles[dd][:, t_sl],
                            rhs=wo_s[:, dd, c_sl],
                            start=(dd == 0),
                            stop=(dd == DC - 1),
                        )
                    nc.scalar.copy(y_sb[:, c_sl], y_ps[:])
                row0 = ch * TC + tb * 128
                nc.sync.dma_start(y_e[row0 : row0 + 128, :], y_sb[:])

    return nc


_np_mm = {F32: np.float32, BF16: np.float32}  # host arrays stay f32; see below


def _mm_np(a, mm_dt):
    if mm_dt == BF16:
        import ml_dtypes

        return np.asarray(a, dtype=ml_dtypes.bfloat16)
    return np.asarray(a, dtype=np.float32)


_cached = {}


def _get_nc(mm_dt):
    key = (T, C, DL, TC, mm_dt)
    if key not in _cached:
        _cached[key] = build_rwkv_nc(T, C, DL, TC, mm_dt=mm_dt, num_devices=8)
    return _cached[key]


def kernel(x, time_decay, time_first, Wk, Wv, Wr, Wo, ln_w, ln_b):
    from concourse.bass_utils import run_bass_kernel_spmd

    x = np.asarray(x, dtype=np.float32)
    time_decay = np.asarray(time_decay, dtype=np.float32)
    time_first = np.asarray(time_first, dtype=np.float32)
    Wk = np.asarray(Wk, dtype=np.float32)
    Wv = np.asarray(Wv, dtype=np.float32)
    Wr = np.asarray(Wr, dtype=np.float32)
    Wo = np.asarray(Wo, dtype=np.float32)
    ln_w = np.asarray(ln_w, dtype=np.float32)
    ln_b = np.asarray(ln_b, dtype=np.float32)
    assert x.shape == (B, T, C)
    if np.any(ln_b != 0.0):
        raise NotImplementedError("nonzero LayerNorm bias not supported")

    mm_dt = MM_DT
    nc = _get_nc(mm_dt)

    td = -np.exp(time_decay)                  # (C,)
    tf = np.exp(time_first)                   # (C,)
    tpos = np.arange(T, dtype=np.float32)
    WkT = np.ascontiguousarray((Wk * ln_w[None, :]).T)  # (C, C) = (c_in, d)
    WvT = np.ascontiguousarray((Wv * ln_w[None, :]).T)
    WrT = np.ascontiguousarray((Wr * ln_w[None, :]).T)
    WoT = np.ascontiguousarray(Wo.T)                    # (c, d)

    half_maps = []
    with np.errstate(under="ignore"):
        for dh in range(2):
            d0, d1 = dh * DL, (dh + 1) * DL
            half_maps.append(
                {
                    "wmat": np.exp(td[d0:d1, None] * tpos[None, :]).astype(
                        np.float32
                    ),
                    "wkT": _mm_np(WkT[:, d0:d1], mm_dt),
                    "wvT": _mm_np(WvT[:, d0:d1], mm_dt),
                    "wrT": _mm_np(WrT[:, d0:d1], mm_dt),
                    "woT": _mm_np(WoT[d0:d1, :], mm_dt),
                    "tf": np.ascontiguousarray(tf[d0:d1]),
                }
            )
    in_maps = []
    for core in range(8):
        b, dh = core // 2, core % 2
        in_maps.append({"x": np.ascontiguousarray(x[b]), **half_maps[dh]})

    global _last_in_maps
    _last_in_maps = in_maps
    res = run_bass_kernel_spmd(nc, in_maps, list(range(8)))
    y = np.empty((B, T, C), dtype=np.float32)
    for b in range(B):
        y[b] = res.results[2 * b]["y"] + res.results[2 * b + 1]["y"]
    return y
